# revision 38
# baseline (speedup 1.0000x reference)
"""Trainium2 Bass kernel for causal MHA (no KV cache), v4d.

Engine model this version is tuned against (TimelineSim + HW ablations):
ACT is the bottleneck: ~113us of exp element work (17.3M scores/core at
1 elem/cycle/lane, 1.2GHz) plus ~430ns/instruction of HW overhead, so
exp instruction count must stay minimal (144 here; a 288-exp variant
measured +62us).  PE matmul streams (incl. all PV) are fully hidden: a
half-PV ablation measured no speedup.  Multi-core lockstep DMA costs
~14us vs a single-core run.

v4 changes over v3 (252us -> 215us):
  - ACT diet: output-projection psum evacuation moved entirely to DVE
    (was half ACT); the two diagonal-pair exps per chunk merged into one
    contiguous exp each (the pair's second diagonal k-tile writes its
    scores left-shifted so the exp'd range is gap-free).
  - xt staged in a chunk-major DRAM layout [128, T*CH]: each 512-token
    chunk loads with two contiguous 4KB/partition DMAs instead of 8
    strided ones; out rows written with one [128,1024] DMA per q-tile.
  - act-table preload: dummy 1-col exp in the preamble so the ~2.7us exp
    table load overlaps the PE warmup.
  - output projections of chunks 0-3 deferred into chunks 5-7 (early
    chunks are PE-bound, late chunks ACT-bound), with the y^T transpose
    DMA issued pairs ahead of its matmuls so the PE FIFO never stalls on
    DMA latency; last chunk's epilogue interleaves the transpose DMAs.
  - v3's wins kept: k-tile-PAIR scores in one [128,1024] psum per head
    with a single exp; the two heads' tiles ping-pong so ACT never waits
    on the scores round-trip; 2-head K=64 scores matmuls adjacent for PE
    row-group concurrency; V tiles [v0 | ones | v1] give softmax
    denominators via the PV matmul's ones column.

PSUM (8 banks, hard budget): scores 2 banks x 2 heads, PV accumulators
1 x 2, qkv/outproj transients 2.  Double-buffering scores is impossible
in 8 banks; a single-k-tile [128,512] variant that double-buffers was
tried and is much slower (doubled exp instruction count dominates).
"""

import os
import sys

import numpy as np

for _p in ("/opt/trn_rl_repo", "/root/.axon_site/_ro/trn_rl_repo"):
    if os.path.isdir(_p) and _p not in sys.path:
        sys.path.insert(0, _p)

import ml_dtypes

import concourse.bass as bass
import concourse.mybir as mybir
import concourse.tile as tile
from concourse import bacc, masks

BF16 = mybir.dt.bfloat16
F32 = mybir.dt.float32
NPBF16 = ml_dtypes.bfloat16

D_MODEL = 1024
N_HEADS = 16
HEAD_DIM = 64
N_CORES = 8
HEADS_PER_CORE = N_HEADS // N_CORES  # 2
T_FULL = 4096
QW = 512  # q-chunk width


def build_program(T: int = T_FULL, loop_reps: int | None = None,
                  ablate: str | None = None) -> bass.Bass:
    # ablate: timing-only experiment knobs (NEVER set in the grading path):
    #   "exp_half" - exp only the first half of each non-diagonal pair tile
    #   "pv_half"  - emit PV matmuls only for even k-tiles
    assert T % QW == 0
    TT = T // 128          # 128-row t-tiles
    CH = D_MODEL // 128    # c-chunks of the contraction dim
    QC = T // QW           # q chunks
    SPC = QW // 128        # q-subtiles per chunk (4)
    KPC = QW // 128        # new k-tiles per chunk (4)

    nc = bacc.Bacc("TRN2", target_bir_lowering=False, debug=False)

    # xt: chunk-major layout; chunk tch occupies cols [tch*4096, (tch+1)*4096),
    # within a chunk cc-major: col = tch*CH*512 + cc*512 + i,
    # value = x[tch*512 + i, cc*128 + p] for partition p.
    xt = nc.dram_tensor("xt", [128, T * CH], BF16, kind="ExternalInput").ap()
    wqk = nc.dram_tensor("wqk", [128, CH * 256], BF16, kind="ExternalInput").ap()
    wv = nc.dram_tensor("wv", [128, CH * 128], BF16, kind="ExternalInput").ap()
    wo = nc.dram_tensor("wo", [128, D_MODEL], BF16, kind="ExternalInput").ap()
    out = nc.dram_tensor("out_partial", [T, D_MODEL], BF16, kind="ExternalOutput").ap()

    with tile.TileContext(nc) as tc:
        import contextlib
        from collections import deque

        EXP = mybir.ActivationFunctionType.Exp

        with contextlib.ExitStack() as ctx:
            const_pool = ctx.enter_context(tc.tile_pool(name="const", bufs=1))
            xt_pool = ctx.enter_context(tc.tile_pool(name="xt", bufs=1))
            qk_pool = ctx.enter_context(tc.tile_pool(name="qk", bufs=1))
            v_pool = ctx.enter_context(tc.tile_pool(name="v", bufs=1))
            y_pool = ctx.enter_context(tc.tile_pool(name="y", bufs=1))
            pt_pool = ctx.enter_context(tc.tile_pool(name="ptp", bufs=6))
            yt_pool = ctx.enter_context(tc.tile_pool(name="ytp", bufs=12))
            ob_pool = ctx.enter_context(tc.tile_pool(name="obp", bufs=6))
            rec_pool = ctx.enter_context(tc.tile_pool(name="recp", bufs=4))
            # scores: [128,1024] (2 banks) per head, single-buffered -> 4
            ps_s = ctx.enter_context(tc.tile_pool(name="pss", bufs=1, space="PSUM"))
            # PV accumulators: one bank per head -> 2
            ps_av = ctx.enter_context(tc.tile_pool(name="psav", bufs=1, space="PSUM"))
            # projection transients -> 2
            ps_t = ctx.enter_context(tc.tile_pool(name="pst", bufs=2, space="PSUM"))

            if loop_reps:
                ctx.enter_context(tc.For_i(0, loop_reps, 1))

            # --- constants ---
            trimask = const_pool.tile([128, 128], BF16, name="trimask")
            masks.make_upper_triangular(nc, trimask, val=1.0, diag=True)

            # --- x^T + weight loads (chunk-major xt: 1 DMA per chunk) ---
            xt_sb = xt_pool.tile([128, T * CH], BF16, name="xt_all", tag="xt")

            def xt_blk(tch, cc):
                base = tch * CH * 512 + cc * 512
                return xt_sb[:, base:base + 512]

            def xt_ktile(kt, cc):
                # [128, 128] block for k-tile kt in channel chunk cc
                tch, r = kt // KPC, kt % KPC
                base = tch * CH * 512 + cc * 512 + r * 128
                return xt_sb[:, base:base + 128]

            def emit_xt_dma(tch, half=None):
                # halves (4KB/partition each): chunk-0 qkv part0 can start
                # after the first half lands, and the smaller bursts collide
                # less across the 8 lockstepped cores.
                c0 = tch * CH * 512
                hw_ = CH * 256
                halves = (0, 1) if half is None else (half,)
                for hf in halves:
                    a = c0 + hf * hw_
                    nc.sync.dma_start(xt_sb[:, a:a + hw_], xt[:, a:a + hw_])

            wqk_sb = const_pool.tile([128, CH * 256], BF16, name="wqk_sb")
            nc.sync.dma_start(wqk_sb, wqk)
            emit_xt_dma(0)
            emit_xt_dma(1)
            wv_sb = const_pool.tile([128, CH * 128], BF16, name="wv_sb")
            nc.sync.dma_start(wv_sb, wv)
            wo_sb = const_pool.tile([128, D_MODEL], BF16, name="wo_sb")
            nc.sync.dma_start(wo_sb, wo)

            # --- persistent tiles ---
            q_sb = qk_pool.tile([128, T], BF16, name="q_sb", tag="q_sb")
            k_sb = qk_pool.tile([128, T], BF16, name="k_sb", tag="k_sb")
            v_sb = [None] * TT          # [128,129] = [v0 | ones | v1]
            y_sb = []
            for qt in range(TT):
                t = y_pool.tile([128, 128], BF16, name=f"y{qt}", tag=f"y{qt}")
                y_sb.append(t)

            # --- deferred emission helpers ---
            _qkv_boxes = {}

            def emit_qkv_part(g, tch, part):
                key = (g, tch)
                c0 = tch * 512
                if part == 0:
                    _qkv_boxes[key] = ps_t.tile([128, 512], F32,
                                                name=f"qk{g}_{tch}", tag="tr")
                ps = _qkv_boxes[key]
                for cc in range(part * 4, part * 4 + 4):
                    nc.tensor.matmul(
                        ps,
                        lhsT=wqk_sb[:, cc * 256 + g * 128: cc * 256 + g * 128 + 128],
                        rhs=xt_blk(tch, cc),
                        start=(cc == 0),
                        stop=(cc == CH - 1),
                    )
                if part == 1:
                    dst = q_sb if g == 0 else k_sb
                    nc.vector.tensor_copy(dst[:, c0:c0 + 512], ps)
                    del _qkv_boxes[key]

            v_pending = {}
            _v_boxes = {}

            def emit_v_part(kt, part):
                if part == 0:
                    _v_boxes[kt] = ps_t.tile([128, 512], F32,
                                             name=f"vps{kt}", tag="tr")
                vps = _v_boxes[kt]
                for cc in range(part * 4, part * 4 + 4):
                    nc.tensor.matmul(
                        vps[:, 0:128],
                        lhsT=xt_ktile(kt, cc),
                        rhs=wv_sb[:, cc * 128:(cc + 1) * 128],
                        start=(cc == 0),
                        stop=(cc == CH - 1),
                    )
                if part == 1:
                    vt = v_pool.tile([128, 129], BF16, name=f"v{kt}",
                                     tag=f"v{kt}")
                    nc.vector.tensor_copy(vt[:, 0:64], vps[:, 0:64])
                    nc.vector.tensor_copy(vt[:, 65:129], vps[:, 64:128])
                    nc.vector.memset(vt[:, 64:65], 1.0)
                    v_sb[kt] = vt
                    del _v_boxes[kt]

            def emit_v_tile(kt):
                emit_v_part(kt, 0)
                emit_v_part(kt, 1)

            def ensure_v(kt):
                fn = v_pending.pop(kt, None)
                if fn is not None:
                    fn()

            def queue_v(kt):
                v_pending[kt] = lambda: emit_v_tile(kt)
                bg.append(lambda: ensure_v(kt))

            # outproj is split in two phases so the PE never stalls on the
            # y^T transpose DMA: the DMA issues several pairs before the
            # matmuls that consume it.
            _yt_box = {}

            def emit_outproj_dma(qt):
                yt = yt_pool.tile([128, 128], BF16, name=f"yt{qt}", tag="yt")
                nc.sync.dma_start(yt, y_sb[qt], transpose=True)
                _yt_box[qt] = yt

            def emit_outproj_mm(qt):
                yt = _yt_box.pop(qt)
                ob = ob_pool.tile([128, D_MODEL], BF16, name=f"ob{qt}", tag="ob")
                for n2 in range(D_MODEL // 512):
                    ops = ps_t.tile([128, 512], F32, name=f"op{qt}_{n2}", tag="tr")
                    nc.tensor.matmul(
                        ops,
                        lhsT=yt,
                        rhs=wo_sb[:, n2 * 512:(n2 + 1) * 512],
                        start=True,
                        stop=True,
                    )
                    nc.vector.tensor_copy(ob[:, n2 * 512:(n2 + 1) * 512], ops)
                nc.sync.dma_start(out[qt * 128:(qt + 1) * 128, :], ob)

            # --- preamble ---
            if T > 1024:
                emit_xt_dma(2)
            warm = ps_t.tile([128, 512], F32, name="warmps", tag="tr")
            # act-table preload for Exp overlapped with PE warmup
            dummy_exp = const_pool.tile([128, 1], BF16, name="dexp")
            nc.scalar.activation(dummy_exp, trimask[:, 0:1], EXP)
            # 44 warm matmuls ~ 4.7us: HAM warm AND covers xt chunk-0 DMA
            for i in range(44):
                nc.tensor.matmul(warm[:, 0:128], lhsT=trimask, rhs=trimask,
                                 start=True, stop=True)
            for g in range(2):
                for part in range(2):
                    emit_qkv_part(g, 0, part)
            # v-tile 0 is NOT built here: deferring it off the serial
            # preamble (to chunk 0's queue_v path) gets the first scores,
            # and hence the first exp, ~1.7us earlier

            # --- attention: k-tile PAIRS, heads interleaved ---
            bg = deque()
            deferred_op = deque()   # outprojs of early chunks, drained late
            sps_box = {}

            for Q in range(QC):
                nkt = KPC * Q + KPC
                npair = nkt // 2
                c0 = Q * QW
                # outproj scheduling: chunks 1-4 are PE-bound (few exps to
                # hide fixed work), so their predecessors' output projections
                # are pushed into the ACT-bound late chunks.  Transpose DMAs
                # go FIRST in bg (issued pairs ahead); matmuls go LAST.
                op_qts = []
                if Q > 0:
                    qts = [SPC * (Q - 1) + s for s in range(SPC)]
                    if Q <= 4:
                        deferred_op.extend(qts)
                    else:
                        op_qts.extend(qts)
                if Q >= 5:
                    take = len(deferred_op) if Q == QC - 1 else 6
                    for _ in range(min(take, len(deferred_op))):
                        op_qts.append(deferred_op.popleft())
                # spread the y^T transpose DMAs among the other head items
                # instead of bunching all ~10 at the queue head (HWDGE burst
                # collides with the xt prefetch otherwise)
                dma_items = [
                    (lambda qt=qt: emit_outproj_dma(qt)) for qt in op_qts
                ]
                if Q + 3 < QC:
                    bg.append(lambda t=Q + 3: emit_xt_dma(t, 0))
                if dma_items:
                    bg.append(dma_items.pop(0))
                if Q + 3 < QC:
                    bg.append(lambda t=Q + 3: emit_xt_dma(t, 1))
                for kt in range(KPC * Q, KPC * Q + KPC):
                    queue_v(kt)
                    if dma_items:
                        bg.append(dma_items.pop(0))
                bg.extend(dma_items)
                # interleave outproj matmuls between (not inside!) the two
                # qkv generations so neither bunches at one end of the
                # chunk; an op MM between a gen's part0/part1 would steal
                # its rotating psum transient (bufs=2) and corrupt it
                op_items = [
                    (lambda qt=qt: emit_outproj_mm(qt)) for qt in op_qts
                ]
                for g in range(2):
                    if Q + 1 < QC:
                        for part in range(2):
                            bg.append(
                                lambda g=g, t=Q + 1, p=part: emit_qkv_part(g, t, p)
                            )
                    if op_items:
                        bg.append(op_items.pop(0))
                bg.extend(op_items)
                iters = npair
                if Q == QC - 1:
                    iters = max(1, iters // 2)
                bg_total = len(bg)
                bg_emitted = 0

                av = [
                    ps_av.tile([128, 65 * SPC], F32, name=f"av{h}_{Q}",
                               tag=f"av{h}")
                    for h in range(HEADS_PER_CORE)
                ]
                # per-head PV output layout: h0 = [pv(64) | den], h1 = [den | pv(64)]
                # (h1's rhs slice starts at the shared ones column)

                def emit_epilogue(Q=Q, av=av, interleave_dma=False):
                    recs = []
                    for h in range(HEADS_PER_CORE):
                        den_off = 64 if h == 0 else 0
                        rec = rec_pool.tile([128, SPC], F32,
                                            name=f"rec{h}_{Q}", tag="rec")
                        nc.vector.reciprocal(
                            rec,
                            av[h][:, den_off:den_off + 65 * (SPC - 1) + 1:65],
                        )
                        recs.append(rec)
                    for s in range(SPC):
                        qt = SPC * Q + s
                        for h in range(HEADS_PER_CORE):
                            dat_off = 0 if h == 0 else 1
                            nc.vector.tensor_scalar_mul(
                                y_sb[qt][:, h * 64:h * 64 + 64],
                                av[h][:, s * 65 + dat_off:s * 65 + dat_off + 64],
                                recs[h][:, s:s + 1],
                            )
                        if interleave_dma:
                            emit_outproj_dma(qt)

                def emit_scores_pair(Qs, ps):
                    kts_ = (2 * ps, 2 * ps + 1)
                    c0_ = Qs * QW
                    tiles = [
                        ps_s.tile([128, 1024], F32, name=f"s{h}_{Qs}_{ps}",
                                  tag=f"s{h}")
                        for h in range(HEADS_PER_CORE)
                    ]
                    # all scores matmuls adjacent, alternating heads: the
                    # K=64 matmuls land in different PE row groups so one
                    # head's LDWEIGHTS/drain hides under the other's matmul.
                    # A diagonal pair's second k-tile writes left-shifted
                    # (at col 512) so the pair's exp'd region [128*i0,
                    # 1024-128*i1) is contiguous and one exp covers it.
                    for j, kt in enumerate(kts_):
                        i = kt - KPC * Qs
                        off = 128 * i if i > 0 else 0
                        for h in range(HEADS_PER_CORE):
                            hp = h * 64
                            dst = (tiles[h][:, off:512] if j == 0
                                   else tiles[h][:, 512:1024 - off])
                            nc.tensor.matmul(
                                dst,
                                lhsT=k_sb[hp:hp + 64, kt * 128:(kt + 1) * 128],
                                rhs=q_sb[hp:hp + 64, c0_ + off:c0_ + 512],
                                start=True,
                                stop=True,
                            )
                    return tiles

                pend_h = [[], []]   # PV matmuls per head, 1-pair delay
                for p in range(npair):
                    kts = (2 * p, 2 * p + 1)
                    if p == 0 and Q in sps_box:
                        sps = sps_box.pop(Q)
                    else:
                        sps = emit_scores_pair(Q, p)
                    i0 = kts[0] - KPC * Q
                    i1 = kts[1] - KPC * Q
                    pts = []
                    for h in range(HEADS_PER_CORE):
                        sp = sps[h]
                        pt = pt_pool.tile([128, 1024], BF16,
                                          name=f"pt{h}_{Q}_{p}", tag=f"pt{h}")
                        if i0 >= 0:
                            # diagonal pair: one contiguous exp
                            lo = 128 * i0
                            hi = 512 + (512 - 128 * i1)
                            nc.scalar.activation(pt[:, lo:hi], sp[:, lo:hi], EXP)
                        elif ablate == "exp_half":
                            nc.scalar.activation(pt[:, 0:512], sp[:, 0:512], EXP)
                        else:
                            nc.scalar.activation(pt, sp, EXP)
                        pts.append(pt)
                        # PV(p-1) for this head runs on PE under this exp
                        for fn in pend_h[h]:
                            fn()
                        pend_h[h] = []
                    if i0 >= 0:
                        for h in range(HEADS_PER_CORE):
                            # diagonal blocks: j=0 at col 128*i0, j=1 at col 512
                            nc.gpsimd.tensor_mul(
                                pts[h][:, 128 * i0:128 * i0 + 128],
                                pts[h][:, 128 * i0:128 * i0 + 128],
                                trimask,
                            )
                            nc.gpsimd.tensor_mul(
                                pts[h][:, 512:640],
                                pts[h][:, 512:640],
                                trimask,
                            )
                    for kt in kts:
                        ensure_v(kt)
                    for h in range(HEADS_PER_CORE):
                        rhs_off = 0 if h == 0 else 64
                        for j, kt in enumerate(kts):
                            i = kt - KPC * Q
                            if ablate == "pv_half" and (kt % 2 == 1) and kt not in (0, nkt - 1):
                                continue
                            for s in range(max(i, 0), SPC):
                                st = kt == 0 and s == max(i, 0)
                                sp_ = kt == nkt - 1 and s == SPC - 1
                                # a diagonal pair's j=1 tile is left-shifted
                                col = (512 + (s - i) * 128
                                       if (j == 1 and i > 0)
                                       else j * 512 + s * 128)
                                pend_h[h].append(
                                    lambda pt=pts[h], s=s, kt=kt, col=col, h=h,
                                    avt=av[h], ro=rhs_off, st=st, sp_=sp_:
                                    nc.tensor.matmul(
                                        avt[:, s * 65:s * 65 + 65],
                                        lhsT=pt[:, col:col + 128],
                                        rhs=v_sb[kt][:, ro:ro + 65],
                                        start=st,
                                        stop=sp_,
                                    )
                                )
                    while bg and bg_emitted < (p + 1) * bg_total // iters:
                        bg.popleft()()
                        bg_emitted += 1
                while bg:
                    bg.popleft()()
                if Q + 1 < QC:
                    # next chunk's q/k just landed (bg drained): emit its
                    # first scores pair now so ACT has no chunk-boundary gap
                    sps_box[Q + 1] = emit_scores_pair(Q + 1, 0)
                for h in range(HEADS_PER_CORE):
                    for fn in pend_h[h]:
                        fn()
                    pend_h[h] = []
                emit_epilogue(interleave_dma=(Q == QC - 1))
                if Q == QC - 1:
                    for s in range(SPC):
                        emit_outproj_mm(SPC * Q + s)

    nc.compile()
    return nc


def make_in_maps(x, w_qkv, w_out, T: int = T_FULL):
    x = np.asarray(x, dtype=np.float32)
    w_qkv = np.asarray(w_qkv, dtype=np.float32)
    w_out = np.asarray(w_out, dtype=np.float32)
    CH = D_MODEL // 128
    QCH = T // QW
    xm = x.reshape(-1, D_MODEL)[:T]                    # [T, 1024]
    # chunk-major xt: [128, T*CH]; block (tch, cc) holds
    # x[tch*512:(tch+1)*512, cc*128:(cc+1)*128].T
    xb = xm.reshape(QCH, QW, CH, 128)                  # [tch, i, cc, p]
    xt_big = np.ascontiguousarray(
        xb.transpose(3, 0, 2, 1).reshape(128, QCH * CH * QW)
    ).astype(NPBF16)

    Wq = w_qkv[0:D_MODEL] * np.float32(1.0 / np.sqrt(HEAD_DIM))
    Wk = w_qkv[D_MODEL:2 * D_MODEL]
    Wv = w_qkv[2 * D_MODEL:3 * D_MODEL]

    in_maps = []
    for c in range(N_CORES):
        r0 = c * 128
        qk_rows = np.concatenate([Wq[r0:r0 + 128], Wk[r0:r0 + 128]], axis=0)
        qk_t = qk_rows.T.reshape(CH, 128, 256).transpose(1, 0, 2).reshape(128, CH * 256)
        v_rows = Wv[r0:r0 + 128]
        v_t = v_rows.T.reshape(CH, 128, 128).transpose(1, 0, 2).reshape(128, CH * 128)
        wo_t = np.ascontiguousarray(w_out[:, r0:r0 + 128].T)
        in_maps.append(
            {
                "xt": xt_big,
                "wqk": np.ascontiguousarray(qk_t).astype(NPBF16),
                "wv": np.ascontiguousarray(v_t).astype(NPBF16),
                "wo": wo_t.astype(NPBF16),
            }
        )
    return in_maps


_program_cache = {}


def get_program(T: int = T_FULL, loop_reps: int | None = None) -> bass.Bass:
    key = (T, loop_reps)
    if key not in _program_cache:
        _program_cache[key] = build_program(T, loop_reps)
    return _program_cache[key]


def run_on_hw(x, w_qkv, w_out, trace: bool = False, T: int = T_FULL):
    from concourse.bass_utils import run_bass_kernel_spmd

    nc = get_program(T)
    in_maps = make_in_maps(x, w_qkv, w_out, T)
    res = run_bass_kernel_spmd(nc, in_maps, core_ids=list(range(N_CORES)), trace=trace)
    acc = np.zeros((T, D_MODEL), np.float32)
    for c in range(N_CORES):
        acc += np.asarray(res.results[c]["out_partial"], dtype=np.float32)
    return acc.reshape(1, T, D_MODEL), res


def kernel(x, w_qkv, w_out):
    out, _ = run_on_hw(x, w_qkv, w_out)
    return out.astype(np.float32)
